# revision 74
# baseline (speedup 1.0000x reference)
"""Trainium2 Bass kernel for ByteLatentEncoder topk_mean_pooling (segment top-4 mean).

Problem: h [8, 4096, 512] f32, patch_ids [8, 4096] int64 (sorted per row,
values in [0, 1024)).  Output [8, 1024, 512]: per (batch, patch, channel),
mean of the top-min(4, count) *distinct* segment values with the reference's
knockout semantics (ties collapse; exhausted ranks contribute exactly -1e9).

Design (data-parallel over batch, one NeuronCore per row; the DVE is the
critical path, so sums run on the idle tensor engine and the DVE only does
the order-statistic part):

  Host repacks h into per-class window tensors (pads pre-baked, 0.25
  prescale baked into B/C values) so the device uses ONLY large direct
  DMAs.  The device writes class-slot-ordered outputs; the host inverts
  the permutation.

  - A (count c <= 4, ~640/row): mean = segment sum = TensorE matmul:
    per 128-patch block, out[p,d] = sum_t W[t,p]*h[t,d] with W[t,p] = 1/c,
    fp8 tokens+weights, <=4 contraction tiles accumulated in one PSUM bank,
    ScalarE-evicted to fp16.  Zero DVE work.
  - B (5 <= c <= 8, ~360/row): top-4-of-8 selection network per q block of
    128 patches (fp16, DVE 2x mode): two Batcher 4-sorts (wide shared
    stage ops) + the cross-max identity top4(a u b) = sum_i max(a_i,
    b_{5-i}).  Blocks are count-descending with per-block plane widths
    (8/6/5) so later blocks skip stages AND bytes.  cmax==5 blocks:
    top4of5 = 0.25*sum5 (TensorE matmul) - min5' (3-op DVE min tree).
    NOTE: gpsimd is left idle on purpose -- it shares SBUF ports with the
    DVE and running tensor ops there stalls the DVE ~1:1.
  - C (c >= 9, ~30/row): slot-major channel-major layout ([P, slot, pair]
    with unit-stride pair ranges, so every network op runs in the DVE 2x
    fp16 mode): three 4-sorts + two cross-max merges, exact for tie-free
    patches.  Fallback (max count > 12): fp32 g-major knockout rank loop.
  - T (tie fixup): host detects patches (c <= 16) with an exact per-channel
    duplicate (sort paths would double-count them).  Those (patch, channel)
    pairs run an exact fp32 knockout rank loop in a tiny [128, TQ, 16+2]
    tile; the host overwrites just those output elements.
"""

import math
from contextlib import ExitStack

import numpy as np

import concourse.bacc as bacc
import concourse.bass as bass
import concourse.mybir as mybir
import concourse.tile as tile
from concourse.bass_utils import run_bass_kernel_spmd

P = 128
SEQ = 4096
DIM = 512
NPATCH = 1024
K = 4
W_A = 4
W_B = 8
W_T = 16
NEGPAD = -1.0e30
CLAMP = -2.5e8  # -1e9/4, clamp for prescaled knockout ranks

C_PERM = [0, 4, 8, 2, 6, 10, 1, 5, 9, 3, 7, 11]

VAL_DT = "fp16"  # B/C value dtype: "f32" | "bf16" | "fp16" (T always fp32-exact)
A_DT = "fp8"     # class-A matmul operand dtype: "fp8" | same-as-VAL_DT

_FLT_MIN = float(np.finfo(np.float32).min)


def _np_dt():
    if VAL_DT == "bf16":
        import ml_dtypes
        return ml_dtypes.bfloat16
    if VAL_DT == "fp16":
        return np.float16
    return np.float32


def _bir_dt():
    return {"bf16": mybir.dt.bfloat16, "fp16": mybir.dt.float16,
            "f32": mybir.dt.float32}[VAL_DT]


def _np_a_dt():
    if A_DT == "fp8":
        import ml_dtypes
        return ml_dtypes.float8_e4m3fn
    return _np_dt()


def _bir_a_dt():
    return mybir.dt.float8e4 if A_DT == "fp8" else _bir_dt()


def _negpad_ab():
    # pad for the A/B value packs -- must be representable in VAL_DT and
    # below any real value (|h|*0.25 << 1e4)
    return -60000.0 if VAL_DT == "fp16" else NEGPAD


def _register_mask_lt():
    """Custom fused DVE op: out = (in0 < in1) ? in0 : -FLT_MAX."""
    from concourse import dve_ops as D
    from concourse.dve_spec import Spec, Src0, Src1, MaxNeg, select, lower, \
        _has_src1
    from concourse.dve_uop import DveOpSpec

    name = "MASK_LT_ANT"
    for op in D.OPS:
        if op.name == name:
            return op

    def _ref(in0, in1, c0, c1, c2):
        a = np.asarray(in0, np.float32)
        b = np.asarray(in1, np.float32).reshape(a.shape)
        return np.where(a < b, a, _FLT_MIN).astype(np.float32)

    spec = Spec(body=select(Src0 < Src1, Src0, MaxNeg), reference=_ref)
    opcode = max(D._SUB_OPCODE_FOR_NAME.values()) + 1
    assert opcode < 0x20
    shas = {}
    for ver in ("v3", "v4"):
        try:
            ds = DveOpSpec(name=name, opcode=opcode, uops=lower(spec, ver=ver),
                           rd1_en=_has_src1(spec))
            shas[ver] = ds.sha(ver)
        except Exception:
            pass
    op = D.DveOp(name, spec, subdim=False, uops_sha=shas)
    D.OPS.append(op)
    D.CUSTOM_DVE_SPECS[name] = spec
    D._SUB_OPCODE_FOR_NAME[name] = opcode
    return op


MASK_LT = _register_mask_lt()


# ---------------------------------------------------------------- host prep

def _row_classes(h_row, pid_row):
    starts = np.searchsorted(pid_row, np.arange(NPATCH + 1)).astype(np.int64)
    counts = np.diff(starts).astype(np.int64)
    starts = starts[:-1]

    # tie detection for c in 2..W_T (covers all classes; the sort paths
    # double-count exact duplicates, so every tie routes to the T fixup)
    ties = []
    sel = np.where((counts >= 2) & (counts <= W_T))[0]
    if len(sel):
        idx = starts[sel, None] + np.arange(W_T)[None, :]
        valid = np.arange(W_T)[None, :] < counts[sel, None]
        idx = np.where(valid, np.minimum(idx, SEQ - 1), 0)
        seg = np.where(valid[:, :, None], h_row[idx], np.inf)
        s = np.sort(seg, axis=1)
        dup = (s[:, 1:, :] == s[:, :-1, :]) & np.isfinite(s[:, 1:, :])
        pi, ch = np.where(dup.any(axis=1))
        ties = [(int(sel[i]), int(c)) for i, c in zip(pi, ch)]

    order = np.argsort(-counts, kind="stable")
    cls_a = [int(p) for p in order if counts[p] <= W_A]
    cls_b = [int(p) for p in order if W_A < counts[p] <= W_B]
    cls_c = [int(p) for p in order if counts[p] > W_B]
    return dict(starts=starts, counts=counts, a=cls_a, b=cls_b, c=cls_c,
                ties=ties, max_c=int(counts.max()))


def _windows(h_row, starts, counts, plist, W):
    """[n, W, DIM] f32 windows; rows j < c are h[start+j], rest NaN-free junk
    marked by the valid mask (returned)."""
    n = len(plist)
    if n == 0:
        return (np.zeros((0, W, DIM), np.float32),
                np.zeros((0, W), bool))
    pl = np.asarray(plist)
    idx = starts[pl][:, None] + np.arange(W)[None, :]
    valid = np.arange(W)[None, :] < counts[pl][:, None]
    idx = np.where(valid, np.minimum(idx, SEQ - 1), 0)
    return h_row[idx], valid


def _part_major(x, Q, width):
    """[Q*P, width] -> [P, Q*width] with slot s=(q*P+r) -> row r, block q."""
    return np.ascontiguousarray(
        x.reshape(Q, P, width).transpose(1, 0, 2).reshape(P, Q * width))


def prepare(h, patch_ids):
    h = np.ascontiguousarray(np.asarray(h, np.float32))
    pid = np.asarray(patch_ids)
    nb = h.shape[0]
    rows = [_row_classes(h[b], pid[b]) for b in range(nb)]

    QA = max(1, math.ceil(max(len(r["a"]) for r in rows) / P))
    QB = max(1, math.ceil(max(len(r["b"]) for r in rows) / P))
    NC = max(len(r["c"]) for r in rows)
    GC = max(1, NC * (DIM // P))  # ceil(NC*512/128)
    WC = max(max(r["max_c"] for r in rows), W_B + 1)
    ntie = max(len(r["ties"]) for r in rows)
    TQ = max(1, math.ceil(ntie / P))
    assert all(r["counts"][p] <= W_T for r in rows for p, _ in r["ties"])

    # static per-q trim level for class B: max count of any slot in
    # block q across rows (blocks are count-descending)
    def q_cmax(key, Q):
        out = np.zeros(Q, np.int64)
        for r in rows:
            cc = r["counts"][r[key]] if len(r[key]) else np.zeros(0, np.int64)
            for q in range(Q):
                seg = cc[q * P:(q + 1) * P]
                if len(seg):
                    out[q] = max(out[q], int(seg.max()))
        return [int(x) for x in out]

    bq_cmax = q_cmax("b", QB)
    # per-block packB width: the cmax>=7 network reads 8 planes, cmax==6
    # reads 6, cmax==5 reads 5 -- don't ship planes nobody reads
    bW = [8 if cm >= 7 else max(int(cm), 5) for cm in bq_cmax]
    boff = [0] * (QB + 1)
    for q in range(QB):
        boff[q + 1] = boff[q] + bW[q]

    # class A now runs on the tensor engine: per 128-patch block q, the mean
    # is a matmul  out[p, d] = sum_t WA[t, p] * h[t, d]  over the block's
    # (<= 128*4 = 512) tokens, with WA[t, p] = 1/c_p.  kt[q] = number of
    # 128-token contraction tiles needed for block q (max across rows).
    # trailing B-blocks with cmax==5 also get a matmul block each, with
    # W = 0.25: top4of5 = 0.25*sum5 - min5'; only the min tree stays on DVE
    # b5/b6 sum blocks FIRST: their msum feeds a DVE op, so they must clear
    # the PE/ScalarE early; the A blocks only feed output DMAs.
    # b6: top4of6 = 0.25*sum6 - bot2'; c5 spill slots use pad B6PAD = -192
    # (1.5*2^7: exact in EVERY fp8 e4m3 flavor -- -256's bit pattern is inf
    # under inf-ful e4m3 -- and fp16 keeps 0.125 granularity at |192|).
    budget = max(0, 8 - QA)
    b5q = [q for q in range(QB) if bq_cmax[q] == 5][:budget]
    b6q = [q for q in range(QB) if bq_cmax[q] == 6][:budget - len(b5q)]
    mm_blocks = [("b5", q) for q in b5q] + [("b6", q) for q in b6q] + \
                [("a", q) for q in range(QA)]
    kt = [0] * len(mm_blocks)
    for r in rows:
        cc = r["counts"]
        for i, (kind, q) in enumerate(mm_blocks):
            key = "a" if kind == "a" else "b"
            pl = r[key][q * P:(q + 1) * P]
            ntok = int(sum(int(cc[p]) for p in pl))
            if kind == "b6":
                ntok += 1  # the bias ones-token
            kt[i] = max(kt[i], (ntok + P - 1) // P)
    kt = [max(k, 1) for k in kt]  # all-zero W tile => zero output row
    ktoff = np.concatenate([[0], np.cumsum(kt)]).astype(int)
    KT = int(ktoff[-1])
    B6PAD = -192.0

    c_sort = WC <= 12 and VAL_DT == "fp16"
    WCP = 12 if c_sort else WC
    dtn = _np_dt()
    in_maps, posts = [], []
    for b, r in enumerate(rows):
        st, cn = r["starts"], r["counts"]

        # matmul inputs: token tiles hA [128, KT*D] (partition = token-in-
        # tile) and weight tiles WA [128, KT*128]; W[t, p] = 1/c_p (class A)
        # or the fixed scale (B5 sum blocks)
        hA = np.zeros((P, KT * DIM), np.float32)
        WA = np.zeros((P, KT * P), np.float32)
        for i, (kind, q) in enumerate(mm_blocks):
            key = "a" if kind == "a" else "b"
            pl = r[key][q * P:(q + 1) * P]
            toks, wcol, winv, bias = [], [], [], []
            for j, p in enumerate(pl):
                c = int(cn[p])
                if c == 0:
                    continue
                toks.extend(range(int(st[p]), int(st[p]) + c))
                wcol.extend([j] * c)
                winv.extend([0.25 if kind != "a" else 1.0 / c] * c)
                if kind == "b6" and c == 5:
                    bias.append(j)
            ntok = len(toks)
            if ntok == 0:
                continue
            hq = np.zeros((kt[i] * P, DIM), np.float32)
            hq[:ntok] = h[b][toks]
            wq = np.zeros((kt[i] * P, P), np.float32)
            wq[np.arange(ntok), wcol] = winv
            if kind == "b6":
                hq[ntok] = 1.0  # ones-token delivers the c5 pad bias
                wq[ntok, bias] = B6PAD
            o = int(ktoff[i])
            hA[:, o * DIM:(o + kt[i]) * DIM] = (
                hq.reshape(kt[i], P, DIM).transpose(1, 0, 2).reshape(P, -1))
            WA[:, o * P:(o + kt[i]) * P] = (
                wq.reshape(kt[i], P, P).transpose(1, 0, 2).reshape(P, -1))
        hA = hA.astype(_np_a_dt())
        WA = WA.astype(_np_a_dt())

        # class B: rows * 0.25, NEGPAD pads; per-block plane width bW[q]
        winB, vB = _windows(h[b], st, cn, r["b"], W_B)
        npad = _negpad_ab()
        winB = np.where(vB[:, :, None], winB * 0.25, npad).astype(np.float32)
        full = np.full((QB * P, W_B, DIM), npad, np.float32)
        full[:len(r["b"])] = winB
        packB = np.empty((P, boff[QB] * DIM), np.float32)
        for q in range(QB):
            blk = full[q * P:(q + 1) * P, :bW[q], :].reshape(P, bW[q] * DIM)
            if q in b6q:
                blk = np.where(blk == npad, B6PAD, blk)  # small pad for bot2
            packB[:, boff[q] * DIM:boff[q + 1] * DIM] = blk
        packB = packB.astype(dtn)

        # class C: channel-major [P, GC*WCP], slot s=(i*512+ch) -> (r=s%P,
        # g=s//P).  Sort path (maxc<=12): fp16, blocks permuted stage-1-ready
        # ([a0,b0,c0,a2,b2,c2 | a1,b1,c1,a3,b3,c3]); else fp32 knockout.
        cpad = _negpad_ab() if c_sort else NEGPAD
        winC, vC = _windows(h[b], st, cn, r["c"], WCP)
        winC = np.where(vC[:, :, None], winC * 0.25, cpad).astype(np.float32)
        if c_sort:
            winC = winC[:, C_PERM, :]
        cvals = winC.transpose(0, 2, 1).reshape(-1, WCP)  # [nC*512, WCP]
        packC = np.full((GC * P, WCP), cpad, np.float32)
        packC[:cvals.shape[0]] = cvals
        if c_sort:
            # slot-major [P, WCP, GC]: every network op runs on a unit-stride
            # [*, GC] range, engaging the DVE 2x fp16 perf mode
            packC = np.ascontiguousarray(
                packC.reshape(GC, P, WCP).transpose(1, 2, 0).reshape(P, WCP * GC))
        else:
            packC = np.ascontiguousarray(
                packC.reshape(GC, P, WCP).transpose(1, 0, 2).reshape(P, GC * WCP))
        packC = packC.astype(np.float16 if c_sort else np.float32)

        # class T: [P, TQ*(W_T+2)] = values*0.25 | scale 4/n | bias (4-n)*1e9/n
        packT = np.full((TQ * P, W_T), NEGPAD, np.float32)
        scaleT = np.zeros((TQ * P, 1), np.float32)
        biasT = np.zeros((TQ * P, 1), np.float32)
        for t, (p, ch) in enumerate(r["ties"]):
            c = int(cn[p])
            n = min(K, c)
            v = h[b][st[p]:st[p] + c, ch] * 0.25
            packT[t, :c] = v
            scaleT[t, 0] = 4.0 / n
            biasT[t, 0] = (K - n) * 1.0e9 / n
        tabT = np.concatenate(
            [packT.reshape(TQ, P, W_T), scaleT.reshape(TQ, P, 1),
             biasT.reshape(TQ, P, 1)], axis=2)
        tabT = np.ascontiguousarray(
            tabT.transpose(1, 0, 2).reshape(P, TQ * (W_T + 2)))

        in_maps.append(dict(hA=np.ascontiguousarray(hA),
                            WA=np.ascontiguousarray(WA),
                            packB=np.ascontiguousarray(packB),
                            packC=packC, tabT=tabT))
        posts.append(r)
    sizes = dict(QA=QA, QB=QB, GC=GC, WC=WCP, TQ=TQ, c_sort=c_sort,
                 bq_cmax=bq_cmax, bW=bW, boff=boff, b5q=b5q, b6q=b6q,
                 kt=kt, ktoff=[int(x) for x in ktoff])
    return in_maps, posts, sizes


# ------------------------------------------------------------- device build

def _ap(t, off, dims):
    a = t[:]
    return bass.AP(a.tensor, a.offset + off, [a.ap[0]] + dims)


def build_kernel(ctx, tc, aps, sizes):
    nc = tc.nc
    dt = mybir.dt
    QA, QB, GC, WC, TQ = (sizes["QA"], sizes["QB"], sizes["GC"], sizes["WC"],
                          sizes["TQ"])
    bq_cmax = sizes["bq_cmax"]
    ddt = _bir_dt()
    D = DIM
    mx, mn, add = (mybir.AluOpType.max, mybir.AluOpType.min,
                   mybir.AluOpType.add)

    kt, ktoff = sizes["kt"], sizes["ktoff"]
    KT = ktoff[-1]
    bW, boff = sizes["bW"], sizes["boff"]
    adt = _bir_a_dt()

    pool = ctx.enter_context(tc.tile_pool(name="main", bufs=1))
    psum = ctx.enter_context(tc.tile_pool(name="psA", bufs=1, space="PSUM"))

    hA = pool.tile([P, KT * D], adt, tag="hA")
    WA = pool.tile([P, KT * P], adt, tag="WA")
    packB = pool.tile([P, boff[QB] * D], ddt, tag="packB")
    cdt = dt.float16 if sizes["c_sort"] else dt.float32
    packC = pool.tile([P, GC * WC], cdt, tag="packC")
    tabT = pool.tile([P, TQ * (W_T + 2)], dt.float32, tag="tabT")
    S1 = pool.tile([P, W_B * D], ddt, tag="S1")
    S2 = pool.tile([P, W_B * D], ddt, tag="S2")
    S3 = pool.tile([P, W_A * D], ddt, tag="S3")
    outA = pool.tile([P, QA * D], ddt, tag="outA")
    outB = pool.tile([P, QB * D], ddt, tag="outB")
    outC = pool.tile([P, GC], dt.float32, tag="outC")
    outT = pool.tile([P, TQ], dt.float32, tag="outT")
    b5q, b6q = sizes["b5q"], sizes["b6q"]
    mm = [("b5", q) for q in b5q] + [("b6", q) for q in b6q] + \
         [("a", q) for q in range(QA)]
    nms = len(b5q) + len(b6q)
    psA = [psum.tile([P, D], dt.float32, tag=f"psA{i}", name=f"psA{i}")
           for i in range(len(mm))]
    if nms:
        msum = pool.tile([P, nms * D], ddt, tag="msum")

    def msum_j(kind, q):
        return (b5q.index(q) if kind == "b5"
                else len(b5q) + b6q.index(q))
    if not sizes["c_sort"]:
        mC = pool.tile([P, GC], dt.float32, tag="mC")
    mT = pool.tile([P, TQ], dt.float32, tag="mT")

    # ---- input DMAs (small first, then in compute order) ----
    # single_packet: fewer descriptors for the two small leading transfers,
    # so they complete before the full DMA-queue set has spun up
    nc.sync.dma_start(tabT[:], aps["tabT"][:], single_packet=True)
    nc.sync.dma_start(packC[:], aps["packC"][:], single_packet=True)
    srcB = aps["packB"][:]

    def dma_bq(q):
        w = bW[q] * D
        nc.sync.dma_start(_ap(packB, boff[q] * D, [[1, w]]),
                          bass.AP(srcB.tensor, srcB.offset + boff[q] * D,
                                  [[boff[QB] * D, P], [1, w]]))

    dma_bq(0)
    # weights/tokens via the Scalar engine's HWDGE queue: their descriptors
    # stream concurrently with bq0's instead of serializing behind it, so
    # the matmul->evict->outA chain closes ~2us earlier
    nc.scalar.dma_start(WA[:], aps["WA"][:])
    nc.scalar.dma_start(hA[:], aps["hA"][:])
    for q in range(1, QB):
        dma_bq(q)

    # ---- exact knockout rank loop on [P, G, W] (stride elems per block) ----
    def knockout_ops(x_t, W, G, stride, m_t, acc_t):
        """Op list (thunks) for the serial knockout chain + the acc AP."""
        x3 = _ap(x_t, 0, [[stride, G], [1, W]])
        m2 = _ap(m_t, 0, [[1, G]])
        m_bc = _ap(m_t, 0, [[1, G], [0, W]])
        acc2 = _ap(acc_t, 0, [[1, G]])
        ops = [
            lambda: nc.vector.tensor_reduce(m2, x3, axis=mybir.AxisListType.X,
                                            op=mx),
            lambda: nc.vector.tensor_scalar_max(acc2, m2, CLAMP),
        ]
        for _ in range(K - 1):
            ops += [
                lambda: nc.vector._custom_dve(MASK_LT, out=x3, in0=x3,
                                              in1=m_bc),
                lambda: nc.vector.tensor_reduce(m2, x3,
                                                axis=mybir.AxisListType.X,
                                                op=mx),
                lambda: nc.vector.scalar_tensor_tensor(out=acc2, in0=m2,
                                                       scalar=CLAMP, in1=acc2,
                                                       op0=mx, op1=add),
            ]
        return ops, acc2

    # class T: tabT block layout [16 vals | scale | bias]
    if sizes["has_t"]:
        t_ops, accT = knockout_ops(tabT, W_T, TQ, W_T + 2, mT, outT)
        for op in t_ops:
            op()
        sc = _ap(tabT, W_T, [[W_T + 2, TQ]])
        bi = _ap(tabT, W_T + 1, [[W_T + 2, TQ]])
        nc.vector.tensor_tensor(accT, accT, sc, op=mybir.AluOpType.mult)
        nc.vector.tensor_tensor(accT, accT, bi, op=add)
        nc.sync.dma_start(aps["outT"][:], outT[:], single_packet=True)

    # class C (emitted interleaved with B q0 below).  Sort path: blocks are
    # three 4-lists in the stage-1-ready C_PERM layout; sort each desc with
    # contiguous-range ops, then cross-max merge a+b, sort the merged top-4,
    # cross-max with c, and sum.  Exact for tie-free patches (ties -> T).
    c_ops = []
    if sizes["has_c"] and sizes["c_sort"]:
        SCc = pool.tile([P, GC * WC], cdt, tag="SCc")
        SDc = pool.tile([P, GC * WC], cdt, tag="SDc")
        SEc = pool.tile([P, GC * WC], cdt, tag="SEc")

        def cs(t, slot, n=1, stride=1):
            # slot-major: slot s occupies the unit-stride range [s*GC, (s+1)*GC)
            if n == 1:
                return _ap(t, slot * GC, [[1, GC]])
            return _ap(t, slot * GC, [[stride * GC, n], [1, GC]])

        def ce(*a, **k):
            c_ops.append(lambda: nc.vector.tensor_tensor(*a, **k))

        ce(cs(SCc, 0, 6), cs(packC, 0, 6), cs(packC, 6, 6), op=mx)  # H
        ce(cs(SCc, 6, 6), cs(packC, 0, 6), cs(packC, 6, 6), op=mn)  # L
        ce(cs(SDc, 0, 3), cs(SCc, 0, 3), cs(SCc, 3, 3), op=mx)  # X1 (rank1s)
        ce(cs(SDc, 3, 3), cs(SCc, 0, 3), cs(SCc, 3, 3), op=mn)  # M1
        ce(cs(SDc, 6, 3), cs(SCc, 6, 3), cs(SCc, 9, 3), op=mx)  # M2
        ce(cs(SDc, 9, 3), cs(SCc, 6, 3), cs(SCc, 9, 3), op=mn)  # X4 (rank4s)
        ce(cs(SEc, 0, 3), cs(SDc, 3, 3), cs(SDc, 6, 3), op=mx)  # X2
        ce(cs(SEc, 3, 3), cs(SDc, 3, 3), cs(SDc, 6, 3), op=mn)  # X3
        # lists desc: a=[SD0,SE0,SE3,SD9], b=+1, c=+2
        # cross a x b-reversed -> m0..m3 @ SE[6..9]
        ce(cs(SEc, 6), cs(SDc, 0), cs(SDc, 10), op=mx)
        ce(cs(SEc, 7), cs(SEc, 0), cs(SEc, 4), op=mx)
        ce(cs(SEc, 8), cs(SEc, 3), cs(SEc, 1), op=mx)
        ce(cs(SEc, 9), cs(SDc, 9), cs(SDc, 1), op=mx)
        # sort4 desc of m0..m3 (positional network on SC[0..3])
        ce(cs(SCc, 0, 2, 2), cs(SEc, 6, 2, 2), cs(SEc, 7, 2, 2), op=mx)
        ce(cs(SCc, 1, 2, 2), cs(SEc, 6, 2, 2), cs(SEc, 7, 2, 2), op=mn)
        ce(cs(SCc, 4, 2, 1), cs(SCc, 0, 2, 1), cs(SCc, 2, 2, 1), op=mx)  # s1,t
        ce(cs(SCc, 6, 2, 1), cs(SCc, 0, 2, 1), cs(SCc, 2, 2, 1), op=mn)  # u,s4
        ce(cs(SCc, 8), cs(SCc, 5), cs(SCc, 6), op=mx)  # s2
        ce(cs(SCc, 9), cs(SCc, 5), cs(SCc, 6), op=mn)  # s3
        # cross M-sorted [SC4,SC8,SC9,SC7] x c-reversed [SD11,SE5,SE2,SD2]
        ce(cs(SCc, 10), cs(SCc, 4), cs(SDc, 11), op=mx)
        ce(cs(SCc, 11), cs(SCc, 8), cs(SEc, 5), op=mx)
        ce(cs(SEc, 10), cs(SCc, 9), cs(SEc, 2), op=mx)
        ce(cs(SEc, 11), cs(SCc, 7), cs(SDc, 2), op=mx)
        ce(cs(SDc, 0, 2, 1), cs(SCc, 10, 2, 1), cs(SEc, 10, 2, 1), op=add)
        ce(_ap(outC, 0, [[1, GC]]), cs(SDc, 0), cs(SDc, 1), op=add)
    elif sizes["has_c"]:
        c_ops, _ = knockout_ops(packC, WC, GC, WC, mC, outC)

    # ---- class B: top4-of-8 selection network per q ----
    def bq_ops(q):
        """Returns the per-q op list as thunks (emission = execution order)."""
        ops = []

        def emit(fn, *a, **k):
            ops.append(lambda: fn(*a, **k))

        cmax = bq_cmax[q]
        IN = boff[q] * D

        def inp(i, npl=1, stride=1):
            return _ap(packB, IN + i * D, [[stride * D, npl], [1, D]])

        def s(t, i, npl=1, stride=1):
            return _ap(t, i * D, [[stride * D, npl], [1, D]])

        def dbl(t, off):
            # planes {off, off+1} u {off+4, off+5} in one wide AP
            return _ap(t, off * D, [[4 * D, 2], [1, 2 * D]])

        if cmax >= 7:
            # both lists sorted DESC with shared wide stage-1/2/3 ops
            emit(nc.vector.tensor_tensor, s(S1, 0, 4, 2), inp(0, 4, 2), inp(1, 4, 2), op=mx)
            emit(nc.vector.tensor_tensor, s(S1, 1, 4, 2), inp(0, 4, 2), inp(1, 4, 2), op=mn)
            emit(nc.vector.tensor_tensor, dbl(S2, 0), dbl(S1, 0), dbl(S1, 2), op=mx)
            emit(nc.vector.tensor_tensor, dbl(S2, 2), dbl(S1, 0), dbl(S1, 2), op=mn)
            emit(nc.vector.tensor_tensor, s(S3, 0, 2, 2), s(S2, 1, 2, 4), s(S2, 2, 2, 4), op=mx)
            emit(nc.vector.tensor_tensor, s(S3, 1, 2, 2), s(S2, 1, 2, 4), s(S2, 2, 2, 4), op=mn)
            # a desc: A1=S2[0], A2=S3[0], A3=S3[1], A4=S2[3]
            # b desc: B1=S2[4], B2=S3[2], B3=S3[3], B4=S2[7]
            # cross pairs: (A1,B4),(A2,B3),(A3,B2),(A4,B1) -> S1[0..3]
            emit(nc.vector.tensor_tensor, s(S1, 0), s(S2, 0), s(S2, 7), op=mx)
            emit(nc.vector.tensor_tensor, s(S1, 1), s(S3, 0), s(S3, 3), op=mx)
            emit(nc.vector.tensor_tensor, s(S1, 2), s(S3, 1), s(S3, 2), op=mx)
            emit(nc.vector.tensor_tensor, s(S1, 3), s(S2, 3), s(S2, 4), op=mx)
            emit(nc.vector.tensor_tensor, s(S1, 4, 2, 1), s(S1, 0, 2, 1), s(S1, 2, 2, 1), op=add)
            emit(nc.vector.tensor_tensor, _ap(outB, q * D, [[1, D]]),
                                    s(S1, 4), s(S1, 5), op=add)
        elif cmax == 6 and q in b6q:
            # top4of6 = msum6 - bot2'.  bot2sum of {v0..v5} = min over the
            # six candidates {s01,s23,s45, n01+n23, n01+n45, n23+n45}; c5
            # slots carry pad=B6PAD in plane 5 and a matching msum bias, so
            # the pad cancels and bot2' degrades to pad + min5'.
            j = msum_j("b6", q)
            emit(nc.vector.tensor_tensor, s(S1, 0, 3, 2), inp(0, 3, 2), inp(1, 3, 2), op=mn)
            emit(nc.vector.tensor_tensor, s(S1, 3, 3, 2), inp(0, 3, 2), inp(1, 3, 2), op=add)
            emit(nc.vector.tensor_copy, s(S1, 6), s(S1, 0))  # n01 dup
            emit(nc.vector.tensor_tensor, s(S2, 0, 2, 1),
                 _ap(S1, 0, [[6 * D, 2], [1, D]]), s(S1, 2, 2, 2), op=add)
            emit(nc.vector.tensor_tensor, s(S2, 2), s(S1, 2), s(S1, 4), op=add)
            emit(nc.vector.tensor_tensor, s(S2, 3, 3, 1), s(S1, 3, 3, 2), s(S2, 0, 3, 1), op=mn)
            emit(nc.vector.tensor_tensor, s(S2, 6), s(S2, 3), s(S2, 4), op=mn)
            emit(nc.vector.tensor_tensor, s(S2, 7), s(S2, 6), s(S2, 5), op=mn)
            emit(nc.vector.tensor_tensor, _ap(outB, q * D, [[1, D]]),
                                    _ap(msum, j * D, [[1, D]]), s(S2, 7),
                                    op=mybir.AluOpType.subtract)
        elif cmax == 6:
            # sort4 (desc) of a-list planes 0..3 only
            emit(nc.vector.tensor_tensor, s(S1, 0, 2, 2), inp(0, 2, 2), inp(1, 2, 2), op=mx)
            emit(nc.vector.tensor_tensor, s(S1, 1, 2, 2), inp(0, 2, 2), inp(1, 2, 2), op=mn)
            emit(nc.vector.tensor_tensor, s(S2, 0, 2, 1), s(S1, 0, 2, 1), s(S1, 2, 2, 1), op=mx)
            emit(nc.vector.tensor_tensor, s(S2, 2, 2, 1), s(S1, 0, 2, 1), s(S1, 2, 2, 1), op=mn)
            emit(nc.vector.tensor_tensor, s(S3, 0), s(S2, 1), s(S2, 2), op=mx)  # A2
            emit(nc.vector.tensor_tensor, s(S3, 1), s(S2, 1), s(S2, 2), op=mn)  # A3
            # A1 = S2[0], A4 = S2[3]

        if cmax == 6 and q not in b6q:
            # b-list: B1 = max(v5,v6), B2 = min, B3 = B4 = NEGPAD
            emit(nc.vector.tensor_tensor, s(S1, 0), inp(4), inp(5), op=mn)  # B2
            emit(nc.vector.tensor_tensor, s(S1, 1), inp(4), inp(5), op=mx)  # B1
            emit(nc.vector.tensor_tensor, s(S1, 2), s(S3, 1), s(S1, 0), op=mx)  # A3|B2
            emit(nc.vector.tensor_tensor, s(S1, 3), s(S2, 3), s(S1, 1), op=mx)  # A4|B1
            emit(nc.vector.tensor_tensor, s(S1, 4), s(S2, 0), s(S3, 0), op=add)  # A1+A2
            emit(nc.vector.tensor_tensor, s(S1, 5), s(S1, 2), s(S1, 3), op=add)
            emit(nc.vector.tensor_tensor, _ap(outB, q * D, [[1, D]]),
                                    s(S1, 4), s(S1, 5), op=add)
        elif cmax <= 5 and q in b5q:
            # cmax == 5: top4of5 = 0.25*sum5 - min5'.  The sum arrives from
            # the tensor engine (msum); only the min tree runs here.
            j = b5q.index(q)
            emit(nc.vector.tensor_tensor, s(S1, 0, 2, 1), inp(0, 2, 1), inp(2, 2, 1), op=mn)
            emit(nc.vector.tensor_tensor, s(S1, 2), s(S1, 0), s(S1, 1), op=mn)
            emit(nc.vector.tensor_tensor, s(S1, 3), s(S1, 2), inp(4), op=mn)  # min5
            emit(nc.vector.tensor_tensor, _ap(outB, q * D, [[1, D]]),
                                    _ap(msum, j * D, [[1, D]]), s(S1, 3),
                                    op=mybir.AluOpType.subtract)
        elif cmax <= 5:
            # fallback without a psum bank: sum5 - min5 on the DVE
            emit(nc.vector.tensor_tensor, s(S1, 0, 2, 1), inp(0, 2, 1), inp(2, 2, 1), op=mn)
            emit(nc.vector.tensor_tensor, s(S1, 2, 2, 1), inp(0, 2, 1), inp(2, 2, 1), op=add)
            emit(nc.vector.tensor_tensor, s(S1, 4), s(S1, 0), s(S1, 1), op=mn)
            emit(nc.vector.tensor_tensor, s(S1, 5), s(S1, 2), s(S1, 3), op=add)
            emit(nc.vector.tensor_tensor, s(S1, 6), s(S1, 4), inp(4), op=mn)  # min5
            emit(nc.vector.tensor_tensor, s(S1, 7), s(S1, 5), inp(4), op=add)  # sum5
            emit(nc.vector.tensor_tensor, _ap(outB, q * D, [[1, D]]),
                                    s(S1, 7), s(S1, 6), op=mybir.AluOpType.subtract)
        return ops

    # interleave class C's ops with B q0's so the vector queue always has
    # work whose data has already arrived (C needs only the small packC)
    q0_ops = bq_ops(0)
    ci = bi = 0
    while ci < len(c_ops) or bi < len(q0_ops):
        if ci * max(len(q0_ops), 1) <= bi * len(c_ops) and ci < len(c_ops):
            c_ops[ci]()
            ci += 1
        elif bi < len(q0_ops):
            q0_ops[bi]()
            bi += 1
        else:
            c_ops[ci]()
            ci += 1
    dstB = aps["outB"][:]
    nc.sync.dma_start(
        bass.AP(dstB.tensor, dstB.offset, [[QB * D, P], [1, D]]),
        _ap(outB, 0, [[1, D]]))
    if sizes["has_c"]:
        nc.sync.dma_start(aps["outC"][:], outC[:], single_packet=True)

    # ---- matmul blocks on the tensor engine: accumulate each block's token
    # tiles into one PSUM bank, then ScalarE-evict to fp16 SBUF ----
    dstA = aps["outA"][:]
    for i, (kind, q) in enumerate(mm):
        o, w = ktoff[i], kt[i]
        for k in range(w):
            nc.tensor.matmul(psA[i][:],
                             _ap(WA, (o + k) * P, [[1, P]]),
                             _ap(hA, (o + k) * D, [[1, D]]),
                             start=(k == 0), stop=(k == w - 1))
        if kind == "a":
            nc.scalar.copy(_ap(outA, q * D, [[1, D]]), psA[i][:])
            nc.sync.dma_start(
                bass.AP(dstA.tensor, dstA.offset + q * D, [[QA * D, P], [1, D]]),
                _ap(outA, q * D, [[1, D]]))
        else:
            nc.scalar.copy(_ap(msum, msum_j(kind, q) * D, [[1, D]]), psA[i][:])

    for q in range(1, QB):
        for op in bq_ops(q):
            op()
        nc.sync.dma_start(
            bass.AP(dstB.tensor, dstB.offset + q * D, [[QB * D, P], [1, D]]),
            _ap(outB, q * D, [[1, D]]))


def build_module(sizes, num_devices):
    nc = bacc.Bacc("TRN2", num_devices=num_devices, debug=False,
                   enable_asserts=False)
    dt = mybir.dt
    ddt = _bir_dt()
    QA, QB, GC, WC, TQ = (sizes["QA"], sizes["QB"], sizes["GC"], sizes["WC"],
                          sizes["TQ"])
    KT = sizes["ktoff"][-1]
    aps = {}
    ins = dict(hA=([P, KT * DIM], _bir_a_dt()),
               WA=([P, KT * P], _bir_a_dt()),
               packB=([P, sizes["boff"][QB] * DIM], ddt),
               packC=([P, GC * WC],
                      dt.float16 if sizes["c_sort"] else dt.float32),
               tabT=([P, TQ * (W_T + 2)], dt.float32))
    outs = dict(outA=([P, QA * DIM], ddt), outB=([P, QB * DIM], ddt),
                outC=([P, GC], dt.float32), outT=([P, TQ], dt.float32))
    for name, (shape, d) in ins.items():
        aps[name] = nc.dram_tensor(name, shape, d, kind="ExternalInput").ap()
    for name, (shape, d) in outs.items():
        aps[name] = nc.dram_tensor(name, shape, d, kind="ExternalOutput").ap()
    with tile.TileContext(nc) as tc:
        with ExitStack() as ctx:
            build_kernel(ctx, tc, aps, sizes)
    nc.compile()
    return nc


# ------------------------------------------------------------ host assembly

def assemble(res, posts, sizes, nb):
    QA, QB, GC, TQ = sizes["QA"], sizes["QB"], sizes["GC"], sizes["TQ"]
    out = np.zeros((nb, NPATCH, DIM), np.float32)
    for b in range(nb):
        r = posts[b]
        d = res.results[b]
        oa = np.asarray(d["outA"], np.float32).reshape(P, QA, DIM)
        oa = oa.transpose(1, 0, 2).reshape(QA * P, DIM)
        out[b][r["a"]] = oa[:len(r["a"])]
        ob = np.asarray(d["outB"], np.float32).reshape(P, QB, DIM)
        ob = ob.transpose(1, 0, 2).reshape(QB * P, DIM)
        out[b][r["b"]] = ob[:len(r["b"])]
        if len(r["c"]):
            oc = np.asarray(d["outC"], np.float32).T.reshape(-1)
            out[b][r["c"]] = oc[:len(r["c"]) * DIM].reshape(len(r["c"]), DIM)
        if len(r["ties"]):
            ot = np.asarray(d["outT"], np.float32).T.reshape(-1)
            for t, (p, ch) in enumerate(r["ties"]):
                out[b][p, ch] = ot[t]
    return out


def _enable_axon_profiling():
    import sys
    import types

    import antenv

    if 'antenv.axon_hooks' not in sys.modules:
        mod = types.ModuleType('antenv.axon_hooks')
        mod._hook = None
        mod.set_axon_ntff_profile_hook = lambda h: setattr(mod, '_hook', h)
        mod.get_axon_ntff_profile_hook = lambda: mod._hook
        sys.modules['antenv.axon_hooks'] = mod
        antenv.axon_hooks = mod
    from antenv import axon_hooks
    if axon_hooks.get_axon_ntff_profile_hook() is None:
        from trn_agent_boot.trn_boot import _ntff_profile_via_ctypes
        axon_hooks.set_axon_ntff_profile_hook(
            _ntff_profile_via_ctypes('/opt/axon/libaxon_pjrt.so'))
    import concourse.bass_utils as bu
    bu.upload_artifacts = lambda tmpdir: tmpdir


def kernel(h, patch_ids, max_num_patches, k, _profile=False):
    assert int(np.asarray(k)) == K
    assert int(np.asarray(max_num_patches)) == NPATCH
    nb = np.asarray(h).shape[0]
    if _profile:
        try:
            _enable_axon_profiling()
        except Exception as e:
            print(f"profiling setup failed ({e}); running without trace")
            _profile = False
    in_maps, posts, sizes = prepare(h, patch_ids)
    sizes["has_c"] = any(len(r["c"]) for r in posts)
    sizes["has_t"] = any(len(r["ties"]) for r in posts)
    nc = build_module(sizes, num_devices=nb)
    res = run_bass_kernel_spmd(nc, in_maps, core_ids=list(range(nb)),
                               trace=_profile)
    out = assemble(res, posts, sizes, nb)
    if _profile:
        kernel.last_results = res
    return out



# revision 75
# speedup vs baseline: 1.2228x; 1.2228x over previous
"""Trainium2 Bass kernel for ByteLatentEncoder topk_mean_pooling (segment top-4 mean).

Problem: h [8, 4096, 512] f32, patch_ids [8, 4096] int64 (sorted per row,
values in [0, 1024)).  Output [8, 1024, 512]: per (batch, patch, channel),
mean of the top-min(4, count) *distinct* segment values with the reference's
knockout semantics (ties collapse; exhausted ranks contribute exactly -1e9).

Design (data-parallel over batch, one NeuronCore per row; the DVE is the
critical path, so sums run on the idle tensor engine and the DVE only does
the order-statistic part):

  Host repacks h into per-class window tensors (pads pre-baked, 0.25
  prescale baked into B/C values) so the device uses ONLY large direct
  DMAs.  The device writes class-slot-ordered outputs; the host inverts
  the permutation.

  - A (count c <= 4, ~640/row): mean = segment sum = TensorE matmul:
    per 128-patch block, out[p,d] = sum_t W[t,p]*h[t,d] with W[t,p] = 1/c,
    fp8 tokens+weights, <=4 contraction tiles accumulated in one PSUM bank,
    ScalarE-evicted to fp16.  Zero DVE work.
  - B (5 <= c <= 8, ~360/row): top-4-of-8 selection network per q block of
    128 patches (fp16, DVE 2x mode): two Batcher 4-sorts (wide shared
    stage ops) + the cross-max identity top4(a u b) = sum_i max(a_i,
    b_{5-i}).  Blocks are count-descending with per-block plane widths
    (8/6/5) so later blocks skip stages AND bytes.  cmax==5 blocks:
    top4of5 = 0.25*sum5 (TensorE matmul) - min5' (3-op DVE min tree).
    NOTE: gpsimd is left idle on purpose -- it shares SBUF ports with the
    DVE and running tensor ops there stalls the DVE ~1:1.
  - C (c >= 9, ~30/row): slot-major channel-major layout ([P, slot, pair]
    with unit-stride pair ranges, so every network op runs in the DVE 2x
    fp16 mode): three 4-sorts + two cross-max merges, exact for tie-free
    patches.  Fallback (max count > 12): fp32 g-major knockout rank loop.
  - T (tie fixup): host detects patches (c <= 16) with an exact per-channel
    duplicate (sort paths would double-count them).  Those (patch, channel)
    pairs run an exact fp32 knockout rank loop in a tiny [128, TQ, 16+2]
    tile; the host overwrites just those output elements.
"""

import math
from contextlib import ExitStack

import numpy as np

import concourse.bacc as bacc
import concourse.bass as bass
import concourse.mybir as mybir
import concourse.tile as tile
from concourse.bass_utils import run_bass_kernel_spmd

P = 128
SEQ = 4096
DIM = 512
NPATCH = 1024
K = 4
W_A = 4
W_B = 8
W_T = 16
NEGPAD = -1.0e30
CLAMP = -2.5e8  # -1e9/4, clamp for prescaled knockout ranks

C_PERM = [0, 4, 8, 2, 6, 10, 1, 5, 9, 3, 7, 11]

VAL_DT = "fp16"  # B/C value dtype: "f32" | "bf16" | "fp16" (T always fp32-exact)
A_DT = "fp8"     # class-A matmul operand dtype: "fp8" | same-as-VAL_DT

_FLT_MIN = float(np.finfo(np.float32).min)


def _np_dt():
    if VAL_DT == "bf16":
        import ml_dtypes
        return ml_dtypes.bfloat16
    if VAL_DT == "fp16":
        return np.float16
    return np.float32


def _bir_dt():
    return {"bf16": mybir.dt.bfloat16, "fp16": mybir.dt.float16,
            "f32": mybir.dt.float32}[VAL_DT]


def _np_a_dt():
    if A_DT == "fp8":
        import ml_dtypes
        return ml_dtypes.float8_e4m3fn
    return _np_dt()


def _bir_a_dt():
    return mybir.dt.float8e4 if A_DT == "fp8" else _bir_dt()


def _negpad_ab():
    # pad for the A/B value packs -- must be representable in VAL_DT and
    # below any real value (|h|*0.25 << 1e4)
    return -60000.0 if VAL_DT == "fp16" else NEGPAD


def _register_mask_lt():
    """Custom fused DVE op: out = (in0 < in1) ? in0 : -FLT_MAX."""
    from concourse import dve_ops as D
    from concourse.dve_spec import Spec, Src0, Src1, MaxNeg, select, lower, \
        _has_src1
    from concourse.dve_uop import DveOpSpec

    name = "MASK_LT_ANT"
    for op in D.OPS:
        if op.name == name:
            return op

    def _ref(in0, in1, c0, c1, c2):
        a = np.asarray(in0, np.float32)
        b = np.asarray(in1, np.float32).reshape(a.shape)
        return np.where(a < b, a, _FLT_MIN).astype(np.float32)

    spec = Spec(body=select(Src0 < Src1, Src0, MaxNeg), reference=_ref)
    opcode = max(D._SUB_OPCODE_FOR_NAME.values()) + 1
    assert opcode < 0x20
    shas = {}
    for ver in ("v3", "v4"):
        try:
            ds = DveOpSpec(name=name, opcode=opcode, uops=lower(spec, ver=ver),
                           rd1_en=_has_src1(spec))
            shas[ver] = ds.sha(ver)
        except Exception:
            pass
    op = D.DveOp(name, spec, subdim=False, uops_sha=shas)
    D.OPS.append(op)
    D.CUSTOM_DVE_SPECS[name] = spec
    D._SUB_OPCODE_FOR_NAME[name] = opcode
    return op


MASK_LT = _register_mask_lt()


# ---------------------------------------------------------------- host prep

def _row_classes(h_row, pid_row):
    starts = np.searchsorted(pid_row, np.arange(NPATCH + 1)).astype(np.int64)
    counts = np.diff(starts).astype(np.int64)
    starts = starts[:-1]

    # tie detection for c in 2..W_T (covers all classes; the sort paths
    # double-count exact duplicates, so every tie routes to the T fixup)
    ties = []
    sel = np.where((counts >= 2) & (counts <= W_T))[0]
    if len(sel):
        idx = starts[sel, None] + np.arange(W_T)[None, :]
        valid = np.arange(W_T)[None, :] < counts[sel, None]
        idx = np.where(valid, np.minimum(idx, SEQ - 1), 0)
        seg = np.where(valid[:, :, None], h_row[idx], np.inf)
        s = np.sort(seg, axis=1)
        dup = (s[:, 1:, :] == s[:, :-1, :]) & np.isfinite(s[:, 1:, :])
        pi, ch = np.where(dup.any(axis=1))
        ties = [(int(sel[i]), int(c)) for i, c in zip(pi, ch)]

    order = np.argsort(-counts, kind="stable")
    cls_a = [int(p) for p in order if counts[p] <= W_A]
    cls_b = [int(p) for p in order if W_A < counts[p] <= W_B]
    cls_c = [int(p) for p in order if counts[p] > W_B]
    return dict(starts=starts, counts=counts, a=cls_a, b=cls_b, c=cls_c,
                ties=ties, max_c=int(counts.max()))


def _windows(h_row, starts, counts, plist, W):
    """[n, W, DIM] f32 windows; rows j < c are h[start+j], rest NaN-free junk
    marked by the valid mask (returned)."""
    n = len(plist)
    if n == 0:
        return (np.zeros((0, W, DIM), np.float32),
                np.zeros((0, W), bool))
    pl = np.asarray(plist)
    idx = starts[pl][:, None] + np.arange(W)[None, :]
    valid = np.arange(W)[None, :] < counts[pl][:, None]
    idx = np.where(valid, np.minimum(idx, SEQ - 1), 0)
    return h_row[idx], valid


def _part_major(x, Q, width):
    """[Q*P, width] -> [P, Q*width] with slot s=(q*P+r) -> row r, block q."""
    return np.ascontiguousarray(
        x.reshape(Q, P, width).transpose(1, 0, 2).reshape(P, Q * width))


def prepare(h, patch_ids):
    h = np.ascontiguousarray(np.asarray(h, np.float32))
    pid = np.asarray(patch_ids)
    nb = h.shape[0]
    rows = [_row_classes(h[b], pid[b]) for b in range(nb)]

    QA = max(1, math.ceil(max(len(r["a"]) for r in rows) / P))
    QB = max(1, math.ceil(max(len(r["b"]) for r in rows) / P))
    NC = max(len(r["c"]) for r in rows)
    GC = max(1, NC * (DIM // P))  # ceil(NC*512/128)
    WC = max(max(r["max_c"] for r in rows), W_B + 1)
    ntie = max(len(r["ties"]) for r in rows)
    TQ = max(1, math.ceil(ntie / P))
    assert all(r["counts"][p] <= W_T for r in rows for p, _ in r["ties"])

    # static per-q trim level for class B: max count of any slot in
    # block q across rows (blocks are count-descending)
    def q_cmax(key, Q):
        out = np.zeros(Q, np.int64)
        for r in rows:
            cc = r["counts"][r[key]] if len(r[key]) else np.zeros(0, np.int64)
            for q in range(Q):
                seg = cc[q * P:(q + 1) * P]
                if len(seg):
                    out[q] = max(out[q], int(seg.max()))
        return [int(x) for x in out]

    bq_cmax = q_cmax("b", QB)
    # per-block packB width: the cmax>=7 network reads 8 planes, cmax==6
    # reads 6, cmax==5 reads 5 -- don't ship planes nobody reads
    bW = [8 if cm >= 7 else max(int(cm), 5) for cm in bq_cmax]
    boff = [0] * (QB + 1)
    for q in range(QB):
        boff[q + 1] = boff[q] + bW[q]

    # class A now runs on the tensor engine: per 128-patch block q, the mean
    # is a matmul  out[p, d] = sum_t WA[t, p] * h[t, d]  over the block's
    # (<= 128*4 = 512) tokens, with WA[t, p] = 1/c_p.  kt[q] = number of
    # 128-token contraction tiles needed for block q (max across rows).
    # trailing B-blocks with cmax==5 also get a matmul block each, with
    # W = 0.25: top4of5 = 0.25*sum5 - min5'; only the min tree stays on DVE
    # b5/b6 sum blocks FIRST: their msum feeds a DVE op, so they must clear
    # the PE/ScalarE early; the A blocks only feed output DMAs.
    # b6: top4of6 = 0.25*sum6 - bot2'; c5 spill slots use pad B6PAD = -192
    # (1.5*2^7: exact in EVERY fp8 e4m3 flavor -- -256's bit pattern is inf
    # under inf-ful e4m3 -- and fp16 keeps 0.125 granularity at |192|).
    budget = max(0, 8 - QA)
    b5q = [q for q in range(QB) if bq_cmax[q] == 5][:budget]
    b6q = [q for q in range(QB) if bq_cmax[q] == 6][:budget - len(b5q)]
    mm_blocks = [("b5", q) for q in b5q] + [("b6", q) for q in b6q] + \
                [("a", q) for q in range(QA)]
    kt = [0] * len(mm_blocks)
    for r in rows:
        cc = r["counts"]
        for i, (kind, q) in enumerate(mm_blocks):
            key = "a" if kind == "a" else "b"
            pl = r[key][q * P:(q + 1) * P]
            ntok = int(sum(int(cc[p]) for p in pl))
            if kind == "b6":
                ntok += 1  # the bias ones-token
            kt[i] = max(kt[i], (ntok + P - 1) // P)
    kt = [max(k, 1) for k in kt]  # all-zero W tile => zero output row
    ktoff = np.concatenate([[0], np.cumsum(kt)]).astype(int)
    KT = int(ktoff[-1])
    B6PAD = -192.0

    c_sort = WC <= 12 and VAL_DT == "fp16"
    WCP = 12 if c_sort else WC
    dtn = _np_dt()
    in_maps, posts = [], []
    for b, r in enumerate(rows):
        st, cn = r["starts"], r["counts"]

        # matmul inputs: token tiles hA [128, KT*D] (partition = token-in-
        # tile) and weight tiles WA [128, KT*128]; W[t, p] = 1/c_p (class A)
        # or the fixed scale (B5 sum blocks)
        hA = np.zeros((P, KT * DIM), np.float32)
        WA = np.zeros((P, KT * P), np.float32)
        for i, (kind, q) in enumerate(mm_blocks):
            key = "a" if kind == "a" else "b"
            pl = r[key][q * P:(q + 1) * P]
            toks, wcol, winv, bias = [], [], [], []
            for j, p in enumerate(pl):
                c = int(cn[p])
                if c == 0:
                    continue
                toks.extend(range(int(st[p]), int(st[p]) + c))
                wcol.extend([j] * c)
                winv.extend([0.25 if kind != "a" else 1.0 / c] * c)
                if kind == "b6" and c == 5:
                    bias.append(j)
            ntok = len(toks)
            if ntok == 0:
                continue
            hq = np.zeros((kt[i] * P, DIM), np.float32)
            hq[:ntok] = h[b][toks]
            wq = np.zeros((kt[i] * P, P), np.float32)
            wq[np.arange(ntok), wcol] = winv
            if kind == "b6":
                hq[ntok] = 1.0  # ones-token delivers the c5 pad bias
                wq[ntok, bias] = B6PAD
            o = int(ktoff[i])
            hA[:, o * DIM:(o + kt[i]) * DIM] = (
                hq.reshape(kt[i], P, DIM).transpose(1, 0, 2).reshape(P, -1))
            WA[:, o * P:(o + kt[i]) * P] = (
                wq.reshape(kt[i], P, P).transpose(1, 0, 2).reshape(P, -1))
        hA = hA.astype(_np_a_dt())
        WA = WA.astype(_np_a_dt())

        # class B: rows * 0.25, NEGPAD pads; per-block plane width bW[q]
        winB, vB = _windows(h[b], st, cn, r["b"], W_B)
        npad = _negpad_ab()
        winB = np.where(vB[:, :, None], winB * 0.25, npad).astype(np.float32)
        full = np.full((QB * P, W_B, DIM), npad, np.float32)
        full[:len(r["b"])] = winB
        packB = np.empty((P, boff[QB] * DIM), np.float32)
        for q in range(QB):
            blk = full[q * P:(q + 1) * P, :bW[q], :].reshape(P, bW[q] * DIM)
            if q in b6q:
                blk = np.where(blk == npad, B6PAD, blk)  # small pad for bot2
            packB[:, boff[q] * DIM:boff[q + 1] * DIM] = blk
        packB = packB.astype(dtn)

        # class C: channel-major [P, GC*WCP], slot s=(i*512+ch) -> (r=s%P,
        # g=s//P).  Sort path (maxc<=12): fp16, blocks permuted stage-1-ready
        # ([a0,b0,c0,a2,b2,c2 | a1,b1,c1,a3,b3,c3]); else fp32 knockout.
        cpad = _negpad_ab() if c_sort else NEGPAD
        winC, vC = _windows(h[b], st, cn, r["c"], WCP)
        winC = np.where(vC[:, :, None], winC * 0.25, cpad).astype(np.float32)
        if c_sort:
            winC = winC[:, C_PERM, :]
        cvals = winC.transpose(0, 2, 1).reshape(-1, WCP)  # [nC*512, WCP]
        packC = np.full((GC * P, WCP), cpad, np.float32)
        packC[:cvals.shape[0]] = cvals
        if c_sort:
            # slot-major [P, WCP, GC]: every network op runs on a unit-stride
            # [*, GC] range, engaging the DVE 2x fp16 perf mode
            packC = np.ascontiguousarray(
                packC.reshape(GC, P, WCP).transpose(1, 2, 0).reshape(P, WCP * GC))
        else:
            packC = np.ascontiguousarray(
                packC.reshape(GC, P, WCP).transpose(1, 0, 2).reshape(P, GC * WCP))
        packC = packC.astype(np.float16 if c_sort else np.float32)

        # class T: [P, TQ*(W_T+2)] = values*0.25 | scale 4/n | bias (4-n)*1e9/n
        packT = np.full((TQ * P, W_T), NEGPAD, np.float32)
        scaleT = np.zeros((TQ * P, 1), np.float32)
        biasT = np.zeros((TQ * P, 1), np.float32)
        for t, (p, ch) in enumerate(r["ties"]):
            c = int(cn[p])
            n = min(K, c)
            v = h[b][st[p]:st[p] + c, ch] * 0.25
            packT[t, :c] = v
            scaleT[t, 0] = 4.0 / n
            biasT[t, 0] = (K - n) * 1.0e9 / n
        tabT = np.concatenate(
            [packT.reshape(TQ, P, W_T), scaleT.reshape(TQ, P, 1),
             biasT.reshape(TQ, P, 1)], axis=2)
        tabT = np.ascontiguousarray(
            tabT.transpose(1, 0, 2).reshape(P, TQ * (W_T + 2)))

        in_maps.append(dict(hA=np.ascontiguousarray(hA),
                            WA=np.ascontiguousarray(WA),
                            packB=np.ascontiguousarray(packB),
                            packC=packC, tabT=tabT))
        posts.append(r)
    sizes = dict(QA=QA, QB=QB, GC=GC, WC=WCP, TQ=TQ, c_sort=c_sort,
                 bq_cmax=bq_cmax, bW=bW, boff=boff, b5q=b5q, b6q=b6q,
                 kt=kt, ktoff=[int(x) for x in ktoff])
    return in_maps, posts, sizes


# ------------------------------------------------------------- device build

def _ap(t, off, dims):
    a = t[:]
    return bass.AP(a.tensor, a.offset + off, [a.ap[0]] + dims)


def build_kernel(ctx, tc, aps, sizes):
    nc = tc.nc
    dt = mybir.dt
    QA, QB, GC, WC, TQ = (sizes["QA"], sizes["QB"], sizes["GC"], sizes["WC"],
                          sizes["TQ"])
    bq_cmax = sizes["bq_cmax"]
    ddt = _bir_dt()
    D = DIM
    mx, mn, add = (mybir.AluOpType.max, mybir.AluOpType.min,
                   mybir.AluOpType.add)

    kt, ktoff = sizes["kt"], sizes["ktoff"]
    KT = ktoff[-1]
    bW, boff = sizes["bW"], sizes["boff"]
    adt = _bir_a_dt()

    pool = ctx.enter_context(tc.tile_pool(name="main", bufs=1))
    psum = ctx.enter_context(tc.tile_pool(name="psA", bufs=1, space="PSUM"))

    hA = pool.tile([P, KT * D], adt, tag="hA")
    WA = pool.tile([P, KT * P], adt, tag="WA")
    packB = pool.tile([P, boff[QB] * D], ddt, tag="packB")
    cdt = dt.float16 if sizes["c_sort"] else dt.float32
    packC = pool.tile([P, GC * WC], cdt, tag="packC")
    tabT = pool.tile([P, TQ * (W_T + 2)], dt.float32, tag="tabT")
    S1 = pool.tile([P, W_B * D], ddt, tag="S1")
    S2 = pool.tile([P, W_B * D], ddt, tag="S2")
    S3 = pool.tile([P, W_A * D], ddt, tag="S3")
    outA = pool.tile([P, QA * D], ddt, tag="outA")
    outB = pool.tile([P, QB * D], ddt, tag="outB")
    outC = pool.tile([P, GC], dt.float32, tag="outC")
    outT = pool.tile([P, TQ], dt.float32, tag="outT")
    b5q, b6q = sizes["b5q"], sizes["b6q"]
    mm = [("b5", q) for q in b5q] + [("b6", q) for q in b6q] + \
         [("a", q) for q in range(QA)]
    nms = len(b5q) + len(b6q)
    psA = [psum.tile([P, D], dt.float32, tag=f"psA{i}", name=f"psA{i}")
           for i in range(len(mm))]
    if nms:
        msum = pool.tile([P, nms * D], ddt, tag="msum")

    def msum_j(kind, q):
        return (b5q.index(q) if kind == "b5"
                else len(b5q) + b6q.index(q))
    if not sizes["c_sort"]:
        mC = pool.tile([P, GC], dt.float32, tag="mC")
    mT = pool.tile([P, TQ], dt.float32, tag="mT")

    # ---- input DMAs (small first, then in compute order) ----
    # single_packet: fewer descriptors for the two small leading transfers,
    # so they complete before the full DMA-queue set has spun up
    nc.sync.dma_start(tabT[:], aps["tabT"][:], single_packet=True)
    nc.sync.dma_start(packC[:], aps["packC"][:], single_packet=True)
    srcB = aps["packB"][:]

    def dma_bq(q):
        w = bW[q] * D
        nc.sync.dma_start(_ap(packB, boff[q] * D, [[1, w]]),
                          bass.AP(srcB.tensor, srcB.offset + boff[q] * D,
                                  [[boff[QB] * D, P], [1, w]]))

    dma_bq(0)
    # weights/tokens next: the b5 matmul chain (matmul -> evict -> q2 sub)
    # must clear early; later packB blocks have slack until the DVE reaches
    # them, and tabT is only needed by the closing T chain
    nc.sync.dma_start(WA[:], aps["WA"][:])
    nc.sync.dma_start(hA[:], aps["hA"][:])
    for q in range(1, QB):
        dma_bq(q)

    # ---- exact knockout rank loop on [P, G, W] (stride elems per block) ----
    def knockout_ops(x_t, W, G, stride, m_t, acc_t):
        """Op list (thunks) for the serial knockout chain + the acc AP."""
        x3 = _ap(x_t, 0, [[stride, G], [1, W]])
        m2 = _ap(m_t, 0, [[1, G]])
        m_bc = _ap(m_t, 0, [[1, G], [0, W]])
        acc2 = _ap(acc_t, 0, [[1, G]])
        ops = [
            lambda: nc.vector.tensor_reduce(m2, x3, axis=mybir.AxisListType.X,
                                            op=mx),
            lambda: nc.vector.tensor_scalar_max(acc2, m2, CLAMP),
        ]
        for _ in range(K - 1):
            ops += [
                lambda: nc.vector._custom_dve(MASK_LT, out=x3, in0=x3,
                                              in1=m_bc),
                lambda: nc.vector.tensor_reduce(m2, x3,
                                                axis=mybir.AxisListType.X,
                                                op=mx),
                lambda: nc.vector.scalar_tensor_tensor(out=acc2, in0=m2,
                                                       scalar=CLAMP, in1=acc2,
                                                       op0=mx, op1=add),
            ]
        return ops, acc2

    # class T: tabT block layout [16 vals | scale | bias]
    if sizes["has_t"]:
        t_ops, accT = knockout_ops(tabT, W_T, TQ, W_T + 2, mT, outT)
        for op in t_ops:
            op()
        sc = _ap(tabT, W_T, [[W_T + 2, TQ]])
        bi = _ap(tabT, W_T + 1, [[W_T + 2, TQ]])
        nc.vector.tensor_tensor(accT, accT, sc, op=mybir.AluOpType.mult)
        nc.vector.tensor_tensor(accT, accT, bi, op=add)
        nc.sync.dma_start(aps["outT"][:], outT[:], single_packet=True)

    # class C (emitted interleaved with B q0 below).  Sort path: blocks are
    # three 4-lists in the stage-1-ready C_PERM layout; sort each desc with
    # contiguous-range ops, then cross-max merge a+b, sort the merged top-4,
    # cross-max with c, and sum.  Exact for tie-free patches (ties -> T).
    c_ops = []
    if sizes["has_c"] and sizes["c_sort"]:
        SCc = pool.tile([P, GC * WC], cdt, tag="SCc")
        SDc = pool.tile([P, GC * WC], cdt, tag="SDc")
        SEc = pool.tile([P, GC * WC], cdt, tag="SEc")

        def cs(t, slot, n=1, stride=1):
            # slot-major: slot s occupies the unit-stride range [s*GC, (s+1)*GC)
            if n == 1:
                return _ap(t, slot * GC, [[1, GC]])
            return _ap(t, slot * GC, [[stride * GC, n], [1, GC]])

        def ce(*a, **k):
            c_ops.append(lambda: nc.vector.tensor_tensor(*a, **k))

        ce(cs(SCc, 0, 6), cs(packC, 0, 6), cs(packC, 6, 6), op=mx)  # H
        ce(cs(SCc, 6, 6), cs(packC, 0, 6), cs(packC, 6, 6), op=mn)  # L
        ce(cs(SDc, 0, 3), cs(SCc, 0, 3), cs(SCc, 3, 3), op=mx)  # X1 (rank1s)
        ce(cs(SDc, 3, 3), cs(SCc, 0, 3), cs(SCc, 3, 3), op=mn)  # M1
        ce(cs(SDc, 6, 3), cs(SCc, 6, 3), cs(SCc, 9, 3), op=mx)  # M2
        ce(cs(SDc, 9, 3), cs(SCc, 6, 3), cs(SCc, 9, 3), op=mn)  # X4 (rank4s)
        ce(cs(SEc, 0, 3), cs(SDc, 3, 3), cs(SDc, 6, 3), op=mx)  # X2
        ce(cs(SEc, 3, 3), cs(SDc, 3, 3), cs(SDc, 6, 3), op=mn)  # X3
        # lists desc: a=[SD0,SE0,SE3,SD9], b=+1, c=+2
        # cross a x b-reversed -> m0..m3 @ SE[6..9]
        ce(cs(SEc, 6), cs(SDc, 0), cs(SDc, 10), op=mx)
        ce(cs(SEc, 7), cs(SEc, 0), cs(SEc, 4), op=mx)
        ce(cs(SEc, 8), cs(SEc, 3), cs(SEc, 1), op=mx)
        ce(cs(SEc, 9), cs(SDc, 9), cs(SDc, 1), op=mx)
        # sort4 desc of m0..m3 (positional network on SC[0..3])
        ce(cs(SCc, 0, 2, 2), cs(SEc, 6, 2, 2), cs(SEc, 7, 2, 2), op=mx)
        ce(cs(SCc, 1, 2, 2), cs(SEc, 6, 2, 2), cs(SEc, 7, 2, 2), op=mn)
        ce(cs(SCc, 4, 2, 1), cs(SCc, 0, 2, 1), cs(SCc, 2, 2, 1), op=mx)  # s1,t
        ce(cs(SCc, 6, 2, 1), cs(SCc, 0, 2, 1), cs(SCc, 2, 2, 1), op=mn)  # u,s4
        ce(cs(SCc, 8), cs(SCc, 5), cs(SCc, 6), op=mx)  # s2
        ce(cs(SCc, 9), cs(SCc, 5), cs(SCc, 6), op=mn)  # s3
        # cross M-sorted [SC4,SC8,SC9,SC7] x c-reversed [SD11,SE5,SE2,SD2]
        ce(cs(SCc, 10), cs(SCc, 4), cs(SDc, 11), op=mx)
        ce(cs(SCc, 11), cs(SCc, 8), cs(SEc, 5), op=mx)
        ce(cs(SEc, 10), cs(SCc, 9), cs(SEc, 2), op=mx)
        ce(cs(SEc, 11), cs(SCc, 7), cs(SDc, 2), op=mx)
        ce(cs(SDc, 0, 2, 1), cs(SCc, 10, 2, 1), cs(SEc, 10, 2, 1), op=add)
        ce(_ap(outC, 0, [[1, GC]]), cs(SDc, 0), cs(SDc, 1), op=add)
    elif sizes["has_c"]:
        c_ops, _ = knockout_ops(packC, WC, GC, WC, mC, outC)

    # ---- class B: top4-of-8 selection network per q ----
    def bq_ops(q):
        """Returns the per-q op list as thunks (emission = execution order)."""
        ops = []

        def emit(fn, *a, **k):
            ops.append(lambda: fn(*a, **k))

        cmax = bq_cmax[q]
        IN = boff[q] * D

        def inp(i, npl=1, stride=1):
            return _ap(packB, IN + i * D, [[stride * D, npl], [1, D]])

        def s(t, i, npl=1, stride=1):
            return _ap(t, i * D, [[stride * D, npl], [1, D]])

        def dbl(t, off):
            # planes {off, off+1} u {off+4, off+5} in one wide AP
            return _ap(t, off * D, [[4 * D, 2], [1, 2 * D]])

        if cmax >= 7:
            # both lists sorted DESC with shared wide stage-1/2/3 ops
            emit(nc.vector.tensor_tensor, s(S1, 0, 4, 2), inp(0, 4, 2), inp(1, 4, 2), op=mx)
            emit(nc.vector.tensor_tensor, s(S1, 1, 4, 2), inp(0, 4, 2), inp(1, 4, 2), op=mn)
            emit(nc.vector.tensor_tensor, dbl(S2, 0), dbl(S1, 0), dbl(S1, 2), op=mx)
            emit(nc.vector.tensor_tensor, dbl(S2, 2), dbl(S1, 0), dbl(S1, 2), op=mn)
            emit(nc.vector.tensor_tensor, s(S3, 0, 2, 2), s(S2, 1, 2, 4), s(S2, 2, 2, 4), op=mx)
            emit(nc.vector.tensor_tensor, s(S3, 1, 2, 2), s(S2, 1, 2, 4), s(S2, 2, 2, 4), op=mn)
            # a desc: A1=S2[0], A2=S3[0], A3=S3[1], A4=S2[3]
            # b desc: B1=S2[4], B2=S3[2], B3=S3[3], B4=S2[7]
            # cross pairs: (A1,B4),(A2,B3),(A3,B2),(A4,B1) -> S1[0..3]
            emit(nc.vector.tensor_tensor, s(S1, 0), s(S2, 0), s(S2, 7), op=mx)
            emit(nc.vector.tensor_tensor, s(S1, 1), s(S3, 0), s(S3, 3), op=mx)
            emit(nc.vector.tensor_tensor, s(S1, 2), s(S3, 1), s(S3, 2), op=mx)
            emit(nc.vector.tensor_tensor, s(S1, 3), s(S2, 3), s(S2, 4), op=mx)
            emit(nc.vector.tensor_tensor, s(S1, 4, 2, 1), s(S1, 0, 2, 1), s(S1, 2, 2, 1), op=add)
            emit(nc.vector.tensor_tensor, _ap(outB, q * D, [[1, D]]),
                                    s(S1, 4), s(S1, 5), op=add)
        elif cmax == 6 and q in b6q:
            # top4of6 = msum6 - bot2'.  bot2sum of {v0..v5} = min over the
            # six candidates {s01,s23,s45, n01+n23, n01+n45, n23+n45}; c5
            # slots carry pad=B6PAD in plane 5 and a matching msum bias, so
            # the pad cancels and bot2' degrades to pad + min5'.
            j = msum_j("b6", q)
            emit(nc.vector.tensor_tensor, s(S1, 0, 3, 2), inp(0, 3, 2), inp(1, 3, 2), op=mn)
            emit(nc.vector.tensor_tensor, s(S1, 3, 3, 2), inp(0, 3, 2), inp(1, 3, 2), op=add)
            emit(nc.vector.tensor_copy, s(S1, 6), s(S1, 0))  # n01 dup
            emit(nc.vector.tensor_tensor, s(S2, 0, 2, 1),
                 _ap(S1, 0, [[6 * D, 2], [1, D]]), s(S1, 2, 2, 2), op=add)
            emit(nc.vector.tensor_tensor, s(S2, 2), s(S1, 2), s(S1, 4), op=add)
            emit(nc.vector.tensor_tensor, s(S2, 3, 3, 1), s(S1, 3, 3, 2), s(S2, 0, 3, 1), op=mn)
            emit(nc.vector.tensor_tensor, s(S2, 6), s(S2, 3), s(S2, 4), op=mn)
            emit(nc.vector.tensor_tensor, s(S2, 7), s(S2, 6), s(S2, 5), op=mn)
            emit(nc.vector.tensor_tensor, _ap(outB, q * D, [[1, D]]),
                                    _ap(msum, j * D, [[1, D]]), s(S2, 7),
                                    op=mybir.AluOpType.subtract)
        elif cmax == 6:
            # sort4 (desc) of a-list planes 0..3 only
            emit(nc.vector.tensor_tensor, s(S1, 0, 2, 2), inp(0, 2, 2), inp(1, 2, 2), op=mx)
            emit(nc.vector.tensor_tensor, s(S1, 1, 2, 2), inp(0, 2, 2), inp(1, 2, 2), op=mn)
            emit(nc.vector.tensor_tensor, s(S2, 0, 2, 1), s(S1, 0, 2, 1), s(S1, 2, 2, 1), op=mx)
            emit(nc.vector.tensor_tensor, s(S2, 2, 2, 1), s(S1, 0, 2, 1), s(S1, 2, 2, 1), op=mn)
            emit(nc.vector.tensor_tensor, s(S3, 0), s(S2, 1), s(S2, 2), op=mx)  # A2
            emit(nc.vector.tensor_tensor, s(S3, 1), s(S2, 1), s(S2, 2), op=mn)  # A3
            # A1 = S2[0], A4 = S2[3]

        if cmax == 6 and q not in b6q:
            # b-list: B1 = max(v5,v6), B2 = min, B3 = B4 = NEGPAD
            emit(nc.vector.tensor_tensor, s(S1, 0), inp(4), inp(5), op=mn)  # B2
            emit(nc.vector.tensor_tensor, s(S1, 1), inp(4), inp(5), op=mx)  # B1
            emit(nc.vector.tensor_tensor, s(S1, 2), s(S3, 1), s(S1, 0), op=mx)  # A3|B2
            emit(nc.vector.tensor_tensor, s(S1, 3), s(S2, 3), s(S1, 1), op=mx)  # A4|B1
            emit(nc.vector.tensor_tensor, s(S1, 4), s(S2, 0), s(S3, 0), op=add)  # A1+A2
            emit(nc.vector.tensor_tensor, s(S1, 5), s(S1, 2), s(S1, 3), op=add)
            emit(nc.vector.tensor_tensor, _ap(outB, q * D, [[1, D]]),
                                    s(S1, 4), s(S1, 5), op=add)
        elif cmax <= 5 and q in b5q:
            # cmax == 5: top4of5 = 0.25*sum5 - min5'.  The sum arrives from
            # the tensor engine (msum); only the min tree runs here.
            j = b5q.index(q)
            emit(nc.vector.tensor_tensor, s(S1, 0, 2, 1), inp(0, 2, 1), inp(2, 2, 1), op=mn)
            emit(nc.vector.tensor_tensor, s(S1, 2), s(S1, 0), s(S1, 1), op=mn)
            emit(nc.vector.tensor_tensor, s(S1, 3), s(S1, 2), inp(4), op=mn)  # min5
            emit(nc.vector.tensor_tensor, _ap(outB, q * D, [[1, D]]),
                                    _ap(msum, j * D, [[1, D]]), s(S1, 3),
                                    op=mybir.AluOpType.subtract)
        elif cmax <= 5:
            # fallback without a psum bank: sum5 - min5 on the DVE
            emit(nc.vector.tensor_tensor, s(S1, 0, 2, 1), inp(0, 2, 1), inp(2, 2, 1), op=mn)
            emit(nc.vector.tensor_tensor, s(S1, 2, 2, 1), inp(0, 2, 1), inp(2, 2, 1), op=add)
            emit(nc.vector.tensor_tensor, s(S1, 4), s(S1, 0), s(S1, 1), op=mn)
            emit(nc.vector.tensor_tensor, s(S1, 5), s(S1, 2), s(S1, 3), op=add)
            emit(nc.vector.tensor_tensor, s(S1, 6), s(S1, 4), inp(4), op=mn)  # min5
            emit(nc.vector.tensor_tensor, s(S1, 7), s(S1, 5), inp(4), op=add)  # sum5
            emit(nc.vector.tensor_tensor, _ap(outB, q * D, [[1, D]]),
                                    s(S1, 7), s(S1, 6), op=mybir.AluOpType.subtract)
        return ops

    # interleave class C's ops with B q0's so the vector queue always has
    # work whose data has already arrived (C needs only the small packC)
    q0_ops = bq_ops(0)
    ci = bi = 0
    while ci < len(c_ops) or bi < len(q0_ops):
        if ci * max(len(q0_ops), 1) <= bi * len(c_ops) and ci < len(c_ops):
            c_ops[ci]()
            ci += 1
        elif bi < len(q0_ops):
            q0_ops[bi]()
            bi += 1
        else:
            c_ops[ci]()
            ci += 1
    dstB = aps["outB"][:]
    nc.sync.dma_start(
        bass.AP(dstB.tensor, dstB.offset, [[QB * D, P], [1, D]]),
        _ap(outB, 0, [[1, D]]))
    if sizes["has_c"]:
        nc.sync.dma_start(aps["outC"][:], outC[:], single_packet=True)

    # ---- matmul blocks on the tensor engine: accumulate each block's token
    # tiles into one PSUM bank, then ScalarE-evict to fp16 SBUF ----
    dstA = aps["outA"][:]
    for i, (kind, q) in enumerate(mm):
        o, w = ktoff[i], kt[i]
        for k in range(w):
            nc.tensor.matmul(psA[i][:],
                             _ap(WA, (o + k) * P, [[1, P]]),
                             _ap(hA, (o + k) * D, [[1, D]]),
                             start=(k == 0), stop=(k == w - 1))
        if kind == "a":
            nc.scalar.copy(_ap(outA, q * D, [[1, D]]), psA[i][:])
            nc.sync.dma_start(
                bass.AP(dstA.tensor, dstA.offset + q * D, [[QA * D, P], [1, D]]),
                _ap(outA, q * D, [[1, D]]))
        else:
            nc.scalar.copy(_ap(msum, msum_j(kind, q) * D, [[1, D]]), psA[i][:])

    for q in range(1, QB):
        for op in bq_ops(q):
            op()
        nc.sync.dma_start(
            bass.AP(dstB.tensor, dstB.offset + q * D, [[QB * D, P], [1, D]]),
            _ap(outB, q * D, [[1, D]]))


def build_module(sizes, num_devices):
    nc = bacc.Bacc("TRN2", num_devices=num_devices, debug=False,
                   enable_asserts=False)
    dt = mybir.dt
    ddt = _bir_dt()
    QA, QB, GC, WC, TQ = (sizes["QA"], sizes["QB"], sizes["GC"], sizes["WC"],
                          sizes["TQ"])
    KT = sizes["ktoff"][-1]
    aps = {}
    ins = dict(hA=([P, KT * DIM], _bir_a_dt()),
               WA=([P, KT * P], _bir_a_dt()),
               packB=([P, sizes["boff"][QB] * DIM], ddt),
               packC=([P, GC * WC],
                      dt.float16 if sizes["c_sort"] else dt.float32),
               tabT=([P, TQ * (W_T + 2)], dt.float32))
    outs = dict(outA=([P, QA * DIM], ddt), outB=([P, QB * DIM], ddt),
                outC=([P, GC], dt.float32), outT=([P, TQ], dt.float32))
    for name, (shape, d) in ins.items():
        aps[name] = nc.dram_tensor(name, shape, d, kind="ExternalInput").ap()
    for name, (shape, d) in outs.items():
        aps[name] = nc.dram_tensor(name, shape, d, kind="ExternalOutput").ap()
    with tile.TileContext(nc) as tc:
        with ExitStack() as ctx:
            build_kernel(ctx, tc, aps, sizes)
    nc.compile()
    return nc


# ------------------------------------------------------------ host assembly

def assemble(res, posts, sizes, nb):
    QA, QB, GC, TQ = sizes["QA"], sizes["QB"], sizes["GC"], sizes["TQ"]
    out = np.zeros((nb, NPATCH, DIM), np.float32)
    for b in range(nb):
        r = posts[b]
        d = res.results[b]
        oa = np.asarray(d["outA"], np.float32).reshape(P, QA, DIM)
        oa = oa.transpose(1, 0, 2).reshape(QA * P, DIM)
        out[b][r["a"]] = oa[:len(r["a"])]
        ob = np.asarray(d["outB"], np.float32).reshape(P, QB, DIM)
        ob = ob.transpose(1, 0, 2).reshape(QB * P, DIM)
        out[b][r["b"]] = ob[:len(r["b"])]
        if len(r["c"]):
            oc = np.asarray(d["outC"], np.float32).T.reshape(-1)
            out[b][r["c"]] = oc[:len(r["c"]) * DIM].reshape(len(r["c"]), DIM)
        if len(r["ties"]):
            ot = np.asarray(d["outT"], np.float32).T.reshape(-1)
            for t, (p, ch) in enumerate(r["ties"]):
                out[b][p, ch] = ot[t]
    return out


def _enable_axon_profiling():
    import sys
    import types

    import antenv

    if 'antenv.axon_hooks' not in sys.modules:
        mod = types.ModuleType('antenv.axon_hooks')
        mod._hook = None
        mod.set_axon_ntff_profile_hook = lambda h: setattr(mod, '_hook', h)
        mod.get_axon_ntff_profile_hook = lambda: mod._hook
        sys.modules['antenv.axon_hooks'] = mod
        antenv.axon_hooks = mod
    from antenv import axon_hooks
    if axon_hooks.get_axon_ntff_profile_hook() is None:
        from trn_agent_boot.trn_boot import _ntff_profile_via_ctypes
        axon_hooks.set_axon_ntff_profile_hook(
            _ntff_profile_via_ctypes('/opt/axon/libaxon_pjrt.so'))
    import concourse.bass_utils as bu
    bu.upload_artifacts = lambda tmpdir: tmpdir


def kernel(h, patch_ids, max_num_patches, k, _profile=False):
    assert int(np.asarray(k)) == K
    assert int(np.asarray(max_num_patches)) == NPATCH
    nb = np.asarray(h).shape[0]
    if _profile:
        try:
            _enable_axon_profiling()
        except Exception as e:
            print(f"profiling setup failed ({e}); running without trace")
            _profile = False
    in_maps, posts, sizes = prepare(h, patch_ids)
    sizes["has_c"] = any(len(r["c"]) for r in posts)
    sizes["has_t"] = any(len(r["ties"]) for r in posts)
    nc = build_module(sizes, num_devices=nb)
    res = run_bass_kernel_spmd(nc, in_maps, core_ids=list(range(nb)),
                               trace=_profile)
    out = assemble(res, posts, sizes, nb)
    if _profile:
        kernel.last_results = res
    return out

